# revision 16
# baseline (speedup 1.0000x reference)
"""GAT message-passing kernel for TRN2 (8 NeuronCores, SPMD).

Algorithm (matches the jax reference up to a softmax shift, which cancels):
  proj = src @ W_src.T ; s_src[n,h] = src[n].w_s[h] ; s_trg[n,h] = trg[n].w_t[h]
  score_e = leakyrelu(s_src[si]+s_trg[ti]) ; p_e = exp(score_e - C_OFF)
  out[t,h,:] = sum_{e: ti=t} p_e * proj[si_e,h,:] / (sum p_e + eps)

Sharding: edges sorted by target; core c owns targets [c*TPC,(c+1)*TPC).
Per 128-target window a one-hot matmul segment-sums numerator||denominator
into PSUM. Per-edge rows (proj bf16 | s_src f32) come from a packed 512B-row
node table (built on device in P0) via dma_gather (int16 idx, 32768-row
slabs); per-edge s_trg comes from a core-local 256B-row table (P0b).
"""
import os
import numpy as np
import ml_dtypes

import concourse.bacc as bacc
import concourse.mybir as mybir
import concourse.tile as tile
from concourse.bass_utils import run_bass_kernel_spmd

BF16 = mybir.dt.bfloat16
F32 = mybir.dt.float32
I16 = mybir.dt.int16

NH, FOUT, D = 8, 16, 128
HF = NH * FOUT  # 128
NEG_SLOPE = 0.2
C_OFF = 16.0
SLAB = 32768
ROW = 256          # bf16 slots per node-table row (512B)
MAX_CT = 4         # max tiles (x128 idx) per dma_gather call
NQ = 4             # SWDGE queues

LAST_EXEC_NS = None


def _install_trace_shim():
    """Register the axon NTFF profile hook (missing antenv.axon_hooks shim)."""
    import sys
    import types

    if "antenv.axon_hooks" in sys.modules:
        return True
    try:
        mod = types.ModuleType("antenv.axon_hooks")
        mod._hook = None
        mod.set_axon_ntff_profile_hook = lambda h: setattr(mod, "_hook", h)
        mod.get_axon_ntff_profile_hook = lambda: mod._hook
        from trn_agent_boot.trn_boot import _ntff_profile_via_ctypes

        mod._hook = _ntff_profile_via_ctypes("/opt/axon/libaxon_pjrt.so")
        sys.modules["antenv.axon_hooks"] = mod
        import concourse.bass_utils as bu

        bu.upload_artifacts = lambda tmpdir: tmpdir
        return True
    except Exception:
        return False


def _wrap_idx(v):
    """[ct*128] int array -> [128, ct*8] int16 wrapped+replicated layout."""
    w = np.asarray(v, dtype=np.int16).reshape(-1, 16).T  # [16, ct*8]
    return np.tile(w, (8, 1))


def _chunks(n, size):
    out = []
    k = 0
    while k < n:
        c = min(size, n - k)
        out.append((k, c))
        k += c
    return out


def build_schedule(si, ti, N, ncores):
    """Common SPMD schedule + per-core packed index arrays."""
    TPC = N // ncores
    WPC = (TPC + 127) // 128
    nslabs = (((N + 127) // 128) * 128 + SLAB - 1) // SLAB

    si = np.asarray(si, dtype=np.int64)
    ti = np.asarray(ti, dtype=np.int64)
    core = ti // TPC
    tloc = ti - core * TPC
    w_of = tloc >> 7
    s_of = si >> 15

    counts = np.zeros((ncores, WPC, nslabs), dtype=np.int64)
    np.add.at(counts, (core, w_of, s_of), 1)
    NT = np.ceil(counts.max(axis=0) / 128).astype(np.int64)  # [WPC, nslabs]
    NW = NT.sum(axis=1)
    T_total = int(NW.sum())

    per_core = []
    for c in range(ncores):
        m = core == c
        csi = si[m]
        cw, cs = w_of[m], s_of[m]
        ctl = tloc[m] & 127
        order = np.lexsort((cs, cw))
        csi, cw, cs, ctl = (a[order] for a in (csi, cw, cs, ctl))

        si_loc = np.zeros(T_total * 128, dtype=np.int64)
        g2 = np.zeros(T_total * 128, dtype=np.int64)
        tl = np.full(T_total * 128, 255, dtype=np.int64)

        keys = cw * nslabs + cs
        starts = np.searchsorted(keys, np.arange(WPC * nslabs))
        ends = np.searchsorted(keys, np.arange(WPC * nslabs), side="right")
        toff = np.zeros(WPC + 1, dtype=np.int64)
        toff[1:] = np.cumsum(NW)
        for w in range(WPC):
            kbase = toff[w]
            srun = 0
            for s in range(nslabs):
                a, b = starts[w * nslabs + s], ends[w * nslabs + s]
                cnt = b - a
                slot0 = (kbase + srun) * 128
                region = int(NT[w, s]) * 128
                g2[slot0 : slot0 + region] = w * 128  # dummy: valid local row
                if cnt:
                    si_loc[slot0 : slot0 + cnt] = csi[a:b] - s * SLAB
                    g2[slot0 : slot0 + cnt] = cw[a:b] * 128 + ctl[a:b]
                    tl[slot0 : slot0 + cnt] = ctl[a:b]
                srun += int(NT[w, s])
        per_core.append((si_loc, g2, tl))

    g1calls = []  # per w: list of (slab, k0, ct)
    g2calls = []  # per w: list of (k0, ct)
    for w in range(WPC):
        calls = []
        k = 0
        for s in range(nslabs):
            for (off, ct) in _chunks(int(NT[w, s]), MAX_CT):
                calls.append((s, k + off, ct))
            k += int(NT[w, s])
        g1calls.append(calls)
        g2calls.append(_chunks(int(NW[w]), MAX_CT))

    packed = []
    for c in range(ncores):
        si_loc, g2, tl = per_core[c]
        blocks1, blocks2 = [], []
        toff = 0
        for w in range(WPC):
            for (s, k0, ct) in g1calls[w]:
                blocks1.append(_wrap_idx(si_loc[(toff + k0) * 128 : (toff + k0 + ct) * 128]))
            for (k0, ct) in g2calls[w]:
                blocks2.append(_wrap_idx(g2[(toff + k0) * 128 : (toff + k0 + ct) * 128]))
            toff += int(NW[w])
        idx1 = np.concatenate(blocks1, axis=1) if blocks1 else np.zeros((128, 8), np.int16)
        idx2 = np.concatenate(blocks2, axis=1) if blocks2 else np.zeros((128, 8), np.int16)
        tl_bf = tl.reshape(T_total, 128).T.astype(np.float32).astype(ml_dtypes.bfloat16)
        packed.append((idx1, idx2, tl_bf))

    return dict(TPC=TPC, WPC=WPC, nslabs=nslabs, NT=NT, NW=NW, T_total=T_total,
                g1calls=g1calls, g2calls=g2calls), packed


def build_nc(N, sched):
    WPC, NW, T_total = sched["WPC"], sched["NW"], sched["T_total"]
    g1calls, g2calls = sched["g1calls"], sched["g2calls"]
    NPAD = ((N + 127) // 128) * 128
    NT0 = NPAD // 128
    LROWS = WPC * 128          # local target rows per core
    NT0B = WPC                 # P0b node tiles

    nc = bacc.Bacc("TRN2", target_bir_lowering=False, num_swdge_queues=NQ)
    srcT = nc.declare_dram_parameter("srcT", [128, NPAD], BF16, isOutput=False)
    trgTl = nc.declare_dram_parameter("trgTl", [128, LROWS], BF16, isOutput=False)
    wext = nc.declare_dram_parameter("wext", [128, 144], BF16, isOutput=False)
    iota = nc.declare_dram_parameter("iota", [128, 128], BF16, isOutput=False)
    idx1 = nc.declare_dram_parameter("idx1", [128, max(T_total * 8, 8)], I16, isOutput=False)
    tlp = nc.declare_dram_parameter("tl", [128, max(T_total, 1)], BF16, isOutput=False)
    tlrep = nc.declare_dram_parameter("tlrep", [128, max(T_total * 128, 128)], BF16, isOutput=False)
    iotac = nc.declare_dram_parameter("iotac", [128, 1], F32, isOutput=False)
    outp = nc.declare_dram_parameter("out", [LROWS, HF], F32, isOutput=True)
    nslabs = sched["nslabs"]
    slab_rows = [min(SLAB, NPAD - s * SLAB) for s in range(nslabs)]
    tables = [nc.dram_tensor(f"table{s}", [slab_rows[s], ROW], BF16)
              for s in range(nslabs)]
    strg = nc.dram_tensor("strg", [LROWS, 16], BF16)

    qrr = [0]

    def next_q():
        q = qrr[0]
        qrr[0] = (q + 1) % NQ
        return q

    with tile.TileContext(nc) as tc:
        with tc.tile_pool(name="const", bufs=1) as cp:
            wext_sb = cp.tile([128, 144], BF16)
            nc.sync.dma_start(out=wext_sb[:], in_=wext[:, :])
            iota_sb = cp.tile([128, 128], BF16)
            nc.sync.dma_start(out=iota_sb[:], in_=iota[:, :])
            tl_sb = cp.tile([128, max(T_total, 1)], BF16)
            nc.sync.dma_start(out=tl_sb[:], in_=tlp[:, :])
            idx1_sb = cp.tile([128, max(T_total * 8, 8)], I16)
            nc.sync.dma_start(out=idx1_sb[:], in_=idx1[:, :])
            iotac_sb = cp.tile([128, 1], F32)
            nc.sync.dma_start(out=iotac_sb[:], in_=iotac[:, :])
            cbias = cp.tile([128, 1], F32)
            nc.vector.memset(cbias[:], -C_OFF)

            # ---- P0: packed node table (proj bf16 | s_src f32) ----
            with (
                tc.tile_pool(name="p0in", bufs=3) as p0in,
                tc.tile_pool(name="p0row", bufs=3) as p0row,
                tc.tile_pool(name="p0ps", bufs=3, space="PSUM") as p0ps,
            ):
                GSZ = 3
                j = 0
                while j < NT0:
                    sl = j // 256          # 256 tiles per 32768-row slab
                    g = min(GSZ, NT0 - j, (sl + 1) * 256 - j)
                    j0 = j * 128
                    jl0 = j0 - sl * SLAB   # row offset within the slab tensor
                    s_t = p0in.tile([128, 128 * g], BF16, tag="s", name=f"s{j}")
                    nc.sync.dma_start(out=s_t[:, :], in_=srcT[:, j0 : j0 + 128 * g])
                    psA = p0ps.tile([128, 136 * g], F32, tag="psA", name=f"pa{j}")
                    for k in range(g):
                        nc.tensor.matmul(
                            out=psA[:, k * 136 : (k + 1) * 136],
                            lhsT=s_t[:, k * 128 : (k + 1) * 128],
                            rhs=wext_sb[:, 0:136],
                            start=True, stop=True,
                        )
                    row = p0row.tile([128, g, ROW], BF16, tag="row", name=f"r{j}")
                    psA_r = psA[:].rearrange("p (k c) -> p k c", c=136)
                    nc.vector.memset(row[:, :, HF:ROW], 0.0)
                    nc.vector.tensor_copy(out=row[:, :, 0:HF], in_=psA_r[:, :, 0:HF])
                    row_f32 = row[:].bitcast(F32)  # [128, g, 128]
                    nc.scalar.copy(out=row_f32[:, :, 64:72], in_=psA_r[:, :, 128:136])
                    nc.sync.dma_start(
                        out=tables[sl][jl0 : jl0 + 128 * g, :].rearrange(
                            "(k p) c -> p k c", p=128),
                        in_=row[:, :, :],
                    )
                    j += g

            # ---- P0b: core-local s_trg table ----
            with (
                tc.tile_pool(name="pbin", bufs=3) as pbin,
                tc.tile_pool(name="pbrow", bufs=3) as pbrow,
                tc.tile_pool(name="pbps", bufs=3, space="PSUM") as pbps,
            ):
                GSZ = 3
                j = 0
                while j < NT0B:
                    g = min(GSZ, NT0B - j)
                    j0 = j * 128
                    t_t = pbin.tile([128, 128 * g], BF16, tag="t", name=f"t{j}")
                    nc.sync.dma_start(out=t_t[:, :], in_=trgTl[:, j0 : j0 + 128 * g])
                    psB = pbps.tile([128, 8 * g], F32, tag="psB", name=f"pb{j}")
                    for k in range(g):
                        nc.tensor.matmul(
                            out=psB[:, k * 8 : (k + 1) * 8],
                            lhsT=t_t[:, k * 128 : (k + 1) * 128],
                            rhs=wext_sb[:, 136:144],
                            start=True, stop=True,
                        )
                    rowb = pbrow.tile([128, g, 16], BF16, tag="rowb", name=f"rb{j}")
                    psB_r = psB[:].rearrange("p (k c) -> p k c", c=8)
                    nc.vector.tensor_copy(out=rowb[:, :, 0:8], in_=psB_r)  # hi
                    lob = pbrow.tile([128, g, 8], F32, tag="lob", name=f"lo{j}")
                    nc.vector.tensor_tensor(
                        out=lob[:], in0=psB_r, in1=rowb[:, :, 0:8],
                        op=mybir.AluOpType.subtract,
                    )
                    nc.scalar.copy(out=rowb[:, :, 8:16], in_=lob[:])
                    nc.sync.dma_start(
                        out=strg[j0 : j0 + 128 * g, :].rearrange("(k p) c -> p k c", p=128),
                        in_=rowb[:, :, :],
                    )
                    j += g

            # ---- P1: edge pass, one 128-target window at a time ----
            with (
                tc.tile_pool(name="g1p", bufs=2) as g1p,
                tc.tile_pool(name="trp", bufs=2) as trp,
                tc.tile_pool(name="ohtp", bufs=2) as ohtp,
                tc.tile_pool(name="swp", bufs=2) as swp,
                tc.tile_pool(name="psep", bufs=2, space="PSUM") as psep,
                tc.tile_pool(name="scp", bufs=2) as scp,
                tc.tile_pool(name="whp", bufs=2) as whp,
                tc.tile_pool(name="ohp", bufs=2) as ohp,
                tc.tile_pool(name="pswp", bufs=2, space="PSUM") as pswp,
                tc.tile_pool(name="epi", bufs=3) as epi,
            ):
                c1off = 0
                toff = 0
                maxnw = int(max(NW)) if len(NW) else 1
                for w in range(WPC):
                    nw = int(NW[w])
                    outt = epi.tile([128, HF], F32, tag="outt", name=f"o{w}")
                    if nw == 0:
                        nc.vector.memset(outt[:], 0.0)
                        nc.sync.dma_start(out=outp[w * 128 : (w + 1) * 128, :], in_=outt[:])
                        continue
                    G1 = g1p.tile([128, maxnw, ROW], BF16, tag="g1", name=f"g1_{w}")
                    for (s, k0, ct) in g1calls[w]:
                        nc.gpsimd.dma_gather(
                            G1[:, k0 : k0 + ct, :],
                            tables[s][:, :],
                            idx1_sb[:, c1off : c1off + ct * 8],
                            ct * 128, ct * 128, ROW,
                            queue_num=next_q(),
                        )
                        c1off += ct * 8
                    # s_trg expansion: onehotT (t,e) built from replicated tl
                    # row vs per-partition iota, then a tiny PE matmul against
                    # the window's s_trg slice.
                    sw = swp.tile([128, 16], BF16, tag="sw", name=f"sw{w}")
                    nc.sync.dma_start(out=sw[:], in_=strg[w * 128 : (w + 1) * 128, :])
                    tr = trp.tile([128, maxnw * 128], BF16, tag="tr", name=f"tr{w}")
                    nc.sync.dma_start(
                        out=tr[:, 0 : nw * 128],
                        in_=tlrep[:, toff * 128 : (toff + nw) * 128],
                    )
                    ohT = ohtp.tile([128, maxnw * 128], BF16, tag="ohT", name=f"ohT{w}")
                    nc.vector.tensor_scalar(
                        out=ohT[:, 0 : nw * 128], in0=tr[:, 0 : nw * 128],
                        scalar1=iotac_sb[:, 0:1], scalar2=None,
                        op0=mybir.AluOpType.is_equal,
                    )
                    pse = psep.tile([128, maxnw * 16], F32, tag="pse", name=f"pse{w}")
                    for k in range(nw):
                        nc.tensor.matmul(
                            out=pse[:, k * 16 : (k + 1) * 16],
                            lhsT=ohT[:, k * 128 : (k + 1) * 128],
                            rhs=sw[:], start=True, stop=True,
                        )
                    G1f = G1[:].bitcast(F32)  # [128, maxnw, 128]
                    pse_r = pse[:].rearrange("p (w c) -> p w c", c=16)
                    sc0 = scp.tile([128, maxnw, 8], F32, tag="sc0", name=f"sc0_{w}")
                    nc.vector.tensor_tensor(
                        out=sc0[:, 0:nw, :], in0=G1f[:, 0:nw, 64:72],
                        in1=pse_r[:, 0:nw, 0:8], op=mybir.AluOpType.add,
                    )
                    sc = scp.tile([128, maxnw, 8], F32, tag="sc", name=f"sc{w}")
                    nc.vector.tensor_tensor(
                        out=sc[:, 0:nw, :], in0=sc0[:, 0:nw, :],
                        in1=pse_r[:, 0:nw, 8:16], op=mybir.AluOpType.add,
                    )
                    # exp(leakyrelu(s) - C) = max(exp(s - C), exp(0.2*s - C))
                    e1 = scp.tile([128, maxnw, 8], F32, tag="e1", name=f"e1_{w}")
                    nc.scalar.activation(
                        e1[:, 0:nw, :], sc[:, 0:nw, :],
                        mybir.ActivationFunctionType.Exp, bias=cbias[:, 0:1],
                    )
                    e2 = scp.tile([128, maxnw, 8], F32, tag="e2", name=f"e2_{w}")
                    nc.scalar.activation(
                        e2[:, 0:nw, :], sc[:, 0:nw, :],
                        mybir.ActivationFunctionType.Exp, bias=cbias[:, 0:1],
                        scale=NEG_SLOPE,
                    )
                    wt = whp.tile([128, maxnw, 136], BF16, tag="wt", name=f"wt{w}")
                    nc.vector.tensor_tensor(
                        out=wt[:, 0:nw, 128:136], in0=e1[:, 0:nw, :],
                        in1=e2[:, 0:nw, :], op=mybir.AluOpType.max,
                    )
                    oh = ohp.tile([128, maxnw, 128], BF16, tag="oh", name=f"oh{w}")
                    iota_b = iota_sb[:].rearrange("p (o c) -> p o c", o=1).to_broadcast(
                        [128, nw, 128]
                    )
                    tl_b = tl_sb[:, toff : toff + nw].rearrange(
                        "p (w o) -> p w o", o=1
                    ).to_broadcast([128, nw, 128])
                    nc.vector.tensor_tensor(
                        out=oh[:, 0:nw, :], in0=iota_b, in1=tl_b,
                        op=mybir.AluOpType.is_equal,
                    )
                    e_b = wt[:, 0:nw, 128:136].rearrange(
                        "p w (h o) -> p w h o", o=1
                    ).to_broadcast([128, nw, 8, 16])
                    nc.vector.tensor_tensor(
                        out=wt[:, 0:nw, 0:128].rearrange("p w (h f) -> p w h f", f=16),
                        in0=G1[:, 0:nw, 0:128].rearrange("p w (h f) -> p w h f", f=16),
                        in1=e_b, op=mybir.AluOpType.mult,
                    )
                    psw = pswp.tile([128, 136], F32, tag="ps", name=f"ps{w}")
                    for k in range(nw):
                        nc.tensor.matmul(
                            out=psw[:], lhsT=oh[:, k, :], rhs=wt[:, k, :],
                            start=(k == 0), stop=(k == nw - 1),
                        )
                    dn = epi.tile([128, 8], F32, tag="dn", name=f"dn{w}")
                    nc.vector.tensor_scalar_add(out=dn[:], in0=psw[:, 128:136], scalar1=1e-16)
                    rc = epi.tile([128, 8], F32, tag="rc", name=f"rc{w}")
                    nc.vector.reciprocal(out=rc[:], in_=dn[:])
                    rc_b = rc[:].rearrange("p (h o) -> p h o", o=1).to_broadcast([128, 8, 16])
                    nc.vector.tensor_tensor(
                        out=outt[:].rearrange("p (h f) -> p h f", f=16),
                        in0=psw[:, 0:HF].rearrange("p (h f) -> p h f", f=16),
                        in1=rc_b, op=mybir.AluOpType.mult,
                    )
                    nc.sync.dma_start(out=outp[w * 128 : (w + 1) * 128, :], in_=outt[:])
                    toff += nw
    nc.compile()
    return nc


def host_prep(trg, src, W_trg, W_src, a_src, a_trg, N, ncores, TPC, WPC):
    NPAD = ((N + 127) // 128) * 128
    LROWS = WPC * 128
    src2 = np.asarray(src, dtype=np.float32).reshape(-1, D)[:N]
    trg2 = np.asarray(trg, dtype=np.float32).reshape(-1, D)[:N]
    W_src = np.asarray(W_src, dtype=np.float32)
    W_trg = np.asarray(W_trg, dtype=np.float32)
    a_src = np.asarray(a_src, dtype=np.float32)
    a_trg = np.asarray(a_trg, dtype=np.float32)
    w_s = np.einsum("hf,hfd->hd", a_src, W_src.reshape(NH, FOUT, D))
    w_t = np.einsum("hf,hfd->hd", a_trg, W_trg.reshape(NH, FOUT, D))
    wext = np.zeros((128, 144), dtype=np.float32)
    wext[:, 0:HF] = W_src.T
    wext[:, HF : HF + 8] = w_s.T
    wext[:, 136:144] = w_t.T
    bf = ml_dtypes.bfloat16
    srcT = np.zeros((128, NPAD), dtype=np.float32)
    srcT[:, :N] = src2.T
    trgTls = []
    for c in range(ncores):
        t = np.zeros((128, LROWS), dtype=np.float32)
        t[:, :TPC] = trg2[c * TPC : (c + 1) * TPC].T
        trgTls.append(t.astype(bf))
    iota = np.tile(np.arange(128, dtype=np.float32), (128, 1))
    return srcT.astype(bf), trgTls, wext.astype(bf), iota.astype(bf)


_CACHE = {}


def run_graph(trg, src, edge_index, W_trg, W_src, a_src, a_trg, N, ncores,
              trace=False):
    global LAST_EXEC_NS
    si = np.asarray(edge_index[0], dtype=np.int64)
    ti = np.asarray(edge_index[1], dtype=np.int64)
    sched, packed = build_schedule(si, ti, N, ncores)
    TPC, WPC, T_total = sched["TPC"], sched["WPC"], sched["T_total"]

    srcT, trgTls, wext, iota = host_prep(
        trg, src, W_trg, W_src, a_src, a_trg, N, ncores, TPC, WPC
    )

    key = (N, ncores, T_total, tuple(int(x) for x in sched["NW"]))
    if key not in _CACHE:
        _CACHE[key] = build_nc(N, sched)
    nc = _CACHE[key]

    in_maps = []
    for c in range(ncores):
        idx1, idx2, tl_bf = packed[c]
        i1 = np.zeros((128, max(T_total * 8, 8)), dtype=np.int16)
        i1[:, : idx1.shape[1]] = idx1
        tlz = np.full((128, max(T_total, 1)), 255.0, dtype=ml_dtypes.bfloat16)
        tlz[:, : tl_bf.shape[1]] = tl_bf
        trep = np.zeros((128, max(T_total * 128, 128)), dtype=ml_dtypes.bfloat16)
        flat = np.ascontiguousarray(tlz[:, :T_total].T).reshape(1, -1)
        trep[:, : T_total * 128] = np.broadcast_to(flat, (128, T_total * 128))
        in_maps.append(
            {"srcT": srcT, "trgTl": trgTls[c], "wext": wext, "iota": iota,
             "idx1": i1, "tl": tlz, "tlrep": trep,
             "iotac": np.arange(128, dtype=np.float32).reshape(128, 1)}
        )

    if trace:
        trace = _install_trace_shim()
    res = run_bass_kernel_spmd(nc, in_maps, core_ids=list(range(ncores)), trace=trace)
    LAST_EXEC_NS = res.exec_time_ns
    out = np.zeros((N, HF), dtype=np.float32)
    for c in range(ncores):
        out[c * TPC : (c + 1) * TPC] = res.results[c]["out"][:TPC]
    return out


def kernel(trg, src, edge_index, W_trg, W_src, a_src, a_trg):
    N = 100000
    out = run_graph(trg, src, edge_index, W_trg, W_src, a_src, a_trg, N, 8,
                    trace=bool(os.environ.get("KERNEL_TRACE")))
    return out.reshape(1, N, HF)
